# revision 10
# baseline (speedup 1.0000x reference)
"""Trainium2 Bass kernel for nn_NNSDecoder (gnn_message_passing).

Reference computation (B=16, N=501, D=128, H=4):
    out[b,i,j] = fc3 . relu(fc2^T relu(feat @ fc1 + b1) + b2) + b3
    feat[b,i,j] = [cp_pre[b,i], cp_post[b,i], cd_pre[b,j], cd_post[b,j]]  (4H=16)

Key algebra: compat[b,n,h] = x[b,n] . (Wk[h] Wq[h]^T q_b), so every
pickup/delivery-side term is linear in h_hat / h_nb rows.  Folding the
head projections and fc1 together gives per-batch 128x32 maps:
    A[b] = h_hat[b] @ G_A1 + h_nb[b] @ G_A2          (N x 32, row/i term)
    C[b] = h_hat[b] @ G_C1 + h_nb[b] @ G_C2          (N x 32, col/j term)
    out[b,i,j] = w3 . relu(W2^T relu(A[b,i] + C[b,j] + b1) + b2) + b3

Device pipeline per batch (i-tiles of 4 rows; pairs of i-tiles):
  - prep: A^T (32 x NP) and C^T stacked 4x (128 x NP) in PSUM;
    crep = C^T + b1 (bf16), a4 = i-tile column layout of A (f32).
  - per pair (t0,t1): X(t) = relu(crep + a4[:,t]) on DVE (bf16 SBUF,
    4x perf mode), fc2 block-diag bf16 matmuls into the two halves of a
    2-bank PSUM tile, one paired Y = relu(pz2 + b2) (ScalarE/DVE split),
    fc3 column-position-packed matmuls into po2 (2 banks = 8 tiles).
  - per po2 (32 out rows): one ScalarE copy to SBUF, one 32-row DMA
    (output DRAM padded to 512 rows so no edge cases).

Sharding: batch dim 16 -> 8 cores x 2 batches (data parallel, weights
replicated). Full inputs in, full output out.
"""

import numpy as np

B, N, D, H = 16, 501, 128, 4
NCORES = 8
BPC = B // NCORES  # batches per core
NP = 504  # padded N: multiple of 8, fits one PSUM bank (<=512 f32)
NT = NP // 4  # 126 i-tiles of 4 rows each
PB = 512  # PSUM bank width in f32; pair tiles use bank-aligned halves
NPAIR = NT // 2  # 63 pairs of i-tiles
NOUT = 512  # padded output rows (uniform 32-row DMAs)

# Y-pair engine split: pairs with (p % 8) < Y_DVE_MOD go to DVE, rest ScalarE
Y_DVE_MOD = 3

_cache = {}


def _build_program():
    import concourse.bacc as bacc
    import concourse.mybir as mybir
    from concourse.tile import TileContext
    from concourse.bass_types import AP

    F32 = mybir.dt.float32
    BF16 = mybir.dt.bfloat16
    nc = bacc.Bacc("TRN2", target_bir_lowering=False, debug=False, num_devices=1)

    hhT = nc.dram_tensor("hhT", [BPC, D, NP], BF16, kind="ExternalInput")
    hnT = nc.dram_tensor("hnT", [BPC, D, NP], BF16, kind="ExternalInput")
    g1a = nc.dram_tensor("g1a", [BPC, D, 32], BF16, kind="ExternalInput")
    g2a = nc.dram_tensor("g2a", [BPC, D, 32], BF16, kind="ExternalInput")
    g1c = nc.dram_tensor("g1c", [BPC, D, 128], BF16, kind="ExternalInput")
    g2c = nc.dram_tensor("g2c", [BPC, D, 128], BF16, kind="ExternalInput")
    w2d = nc.dram_tensor("w2d", [D, 128], BF16, kind="ExternalInput")
    w3d = nc.dram_tensor("w3d", [D, 4], BF16, kind="ExternalInput")
    b1r = nc.dram_tensor("b1r", [D, 1], F32, kind="ExternalInput")
    b2r = nc.dram_tensor("b2r", [D, 1], F32, kind="ExternalInput")
    out = nc.dram_tensor("out", [BPC, NOUT, N], F32, kind="ExternalOutput")

    add = mybir.AluOpType.add
    amax = mybir.AluOpType.max
    Relu = mybir.ActivationFunctionType.Relu

    with TileContext(nc) as tc:
        with (
            tc.tile_pool(name="const", bufs=1) as cpool,
            tc.tile_pool(name="batch", bufs=2) as bpool,
            tc.tile_pool(name="x", bufs=6) as xpool,
            tc.tile_pool(name="y", bufs=4) as ypool,
            tc.tile_pool(name="o", bufs=2) as opool,
            tc.tile_pool(name="pz", bufs=2, space="PSUM") as pzpool,
            tc.tile_pool(name="po", bufs=2, space="PSUM") as popool,
        ):
            w2t = cpool.tile([D, 128], BF16)
            nc.sync.dma_start(w2t[:], w2d.ap()[:, :])
            w3t = cpool.tile([D, 4], BF16)
            nc.sync.dma_start(w3t[:], w3d.ap()[:, :])
            b1t = cpool.tile([D, 1], F32)
            nc.sync.dma_start(b1t[:], b1r.ap()[:, :])
            b2t = cpool.tile([D, 1], F32)
            nc.sync.dma_start(b2t[:], b2r.ap()[:, :])

            for b in range(BPC):
                hh = bpool.tile([D, NP], BF16, tag="hh")
                nc.sync.dma_start(hh[:], hhT.ap()[b, :, :])
                hn = bpool.tile([D, NP], BF16, tag="hn")
                nc.sync.dma_start(hn[:], hnT.ap()[b, :, :])
                g1at = bpool.tile([D, 32], BF16, tag="g1a")
                nc.sync.dma_start(g1at[:], g1a.ap()[b, :, :])
                g2at = bpool.tile([D, 32], BF16, tag="g2a")
                nc.sync.dma_start(g2at[:], g2a.ap()[b, :, :])
                g1ct = bpool.tile([D, 128], BF16, tag="g1c")
                nc.sync.dma_start(g1ct[:], g1c.ap()[b, :, :])
                g2ct = bpool.tile([D, 128], BF16, tag="g2c")
                nc.sync.dma_start(g2ct[:], g2c.ap()[b, :, :])

                # prep PSUM borrowed from the pz pool: one 2-bank tile holds
                # C^T (x4 replicated) in the first half, A^T in the second.
                pprep = pzpool.tile([D, 2 * PB], F32, tag="pz")
                pac = pprep[:, 0:NP]
                paa = pprep[0:32, PB : PB + NP]
                nc.tensor.matmul(paa, g1at[:], hh[:], start=True, stop=False)
                nc.tensor.matmul(paa, g2at[:], hn[:], start=False, stop=True)
                nc.tensor.matmul(pac, g1ct[:], hh[:], start=True, stop=False)
                nc.tensor.matmul(pac, g2ct[:], hn[:], start=False, stop=True)

                # crep = C^T(rep4) + b1   (one op, bias folded in)
                crep = bpool.tile([D, NP], BF16, tag="crep")
                nc.vector.tensor_scalar_add(crep[:], pac, b1t[:, 0:1])

                # a4[32r+k, t] = A^T[k, 4t+r]  (i-tile column layout, f32)
                a4 = bpool.tile([D, NT], F32, tag="a4")
                paa_r = paa.rearrange("p (t r) -> p r t", r=4)
                for r in range(4):
                    nc.scalar.copy(a4[32 * r : 32 * r + 32, :], paa_r[:, r, :])

                po2 = None
                xq = {}

                def emit_x(p):
                    xs = []
                    for t in (2 * p, 2 * p + 1):
                        x = xpool.tile([D, NP], BF16, name=f"x{b}_{t}", tag="x")
                        nc.vector.tensor_scalar(
                            out=x[:],
                            in0=crep[:],
                            scalar1=a4[:, t : t + 1],
                            scalar2=0.0,
                            op0=add,
                            op1=amax,
                        )
                        xs.append(x)
                    xq[p] = xs

                emit_x(0)
                emit_x(1)
                for p in range(NPAIR):
                    t0 = 2 * p
                    if p + 2 < NPAIR:
                        emit_x(p + 2)
                    # fc2 for the two tiles of this pair
                    pz2 = pzpool.tile([D, 2 * PB], F32, tag="pz")
                    for s, x in enumerate(xq.pop(p)):
                        nc.tensor.matmul(
                            pz2[:, s * PB : s * PB + NP],
                            w2t[:],
                            x[:],
                            start=True,
                            stop=True,
                        )
                    # paired Y = relu(pz2 + b2) -> bf16
                    y2 = ypool.tile([D, 2 * PB], BF16)
                    if p % 16 in (0, 3, 6, 10, 13):
                        nc.vector.tensor_scalar(
                            out=y2[:],
                            in0=pz2[:],
                            scalar1=b2t[:, 0:1],
                            scalar2=0.0,
                            op0=add,
                            op1=amax,
                        )
                    else:
                        nc.scalar.activation(
                            y2[:], pz2[:], Relu, bias=b2t[:, 0:1]
                        )
                    # fc3 for both tiles, packed into po2 (8 tiles / po2)
                    for s, t in enumerate((t0, t0 + 1)):
                        u = t % 4
                        q = (t // 4) % 2
                        if t % 8 == 0:
                            po2 = popool.tile([D, 2 * PB], F32)
                        nc.tensor.matmul(
                            po2[32 * u : 32 * u + 4, q * PB : q * PB + NP],
                            w3t[:],
                            y2[:, s * PB : s * PB + NP],
                            start=True,
                            stop=True,
                            tile_position=(0, 32 * u),
                        )
                        if t == NT - 1 or t % 8 == 7:
                            # po2 complete (or end of batch): copy + DMA out
                            ob = opool.tile([D, 2 * PB], F32)
                            nc.scalar.copy(ob[:], po2[:])
                            gi = t // 8  # po2 index; rows 32*gi .. 32*gi+31
                            base = ob[:, :]
                            pitch = base.ap[0][0]  # partition pitch (elements)
                            seg = out.ap()[
                                b, 32 * gi : 32 * gi + 32, :
                            ].rearrange("(q u r) n -> r u q n", q=2, u=4)
                            for r in range(4):
                                src = AP(
                                    base.tensor,
                                    base.offset + r * pitch,
                                    [
                                        [32 * pitch, 4],  # u: partition group
                                        [PB, 2],  # q: column half
                                        [1, N],  # j
                                    ],
                                )
                                nc.sync.dma_start(seg[r], src)

    nc.compile()
    return nc


def _host_prep(h_hat, pos_pickup, pos_delivery, solution, Wq1, Wk1, Wq2, Wk2, fc1_w):
    """Per-batch tiny maps G (128x32 each) + transposed/padded node features."""
    import ml_dtypes

    f32 = np.float32
    bf16 = ml_dtypes.bfloat16
    h_hat = np.asarray(h_hat, f32)
    pp = np.asarray(pos_pickup).astype(np.int64)
    pd = np.asarray(pos_delivery).astype(np.int64)
    sol = np.asarray(solution).astype(np.int64)
    Wq1 = np.asarray(Wq1, f32)
    Wk1 = np.asarray(Wk1, f32)
    Wq2 = np.asarray(Wq2, f32)
    Wk2 = np.asarray(Wk2, f32)
    fc1_w = np.asarray(fc1_w, f32)

    hhT = np.zeros((B, D, NP), bf16)
    hnT = np.zeros((B, D, NP), bf16)
    g1a = np.zeros((B, D, 32), bf16)
    g2a = np.zeros((B, D, 32), bf16)
    g1c = np.zeros((B, D, 128), bf16)
    g2c = np.zeros((B, D, 128), bf16)

    for b in range(B):
        hb = h_hat[b]  # (N, D)
        hnb = hb[sol[b]]  # (N, D) gathered neighbours
        hhT[b, :, :N] = hb.T
        hnT[b, :, :N] = hnb.T
        p = hb[pp[b]]  # (D,)
        dv = hb[pd[b]]
        # u[h] = Wk[h] @ (Wq[h]^T @ q): compat[n,h] = x[n] . u[h]
        U1p = np.stack([Wk1[h] @ (Wq1[h].T @ p) for h in range(H)], axis=1)
        U2p = np.stack([Wk2[h] @ (Wq2[h].T @ p) for h in range(H)], axis=1)
        U1d = np.stack([Wk1[h] @ (Wq1[h].T @ dv) for h in range(H)], axis=1)
        U2d = np.stack([Wk2[h] @ (Wq2[h].T @ dv) for h in range(H)], axis=1)
        g1a[b] = U1p @ fc1_w[0:4]  # h_hat -> A
        g2a[b] = U2p @ fc1_w[4:8]  # h_nb  -> A
        gc1 = U1d @ fc1_w[8:12]  # h_hat -> C
        gc2 = U2d @ fc1_w[12:16]  # h_nb  -> C
        g1c[b] = np.tile(gc1, (1, 4))
        g2c[b] = np.tile(gc2, (1, 4))
    return hhT, hnT, g1a, g2a, g1c, g2c


_last_results = None


def kernel(
    h_hat,
    pos_pickup,
    pos_delivery,
    solution,
    Wq1,
    Wk1,
    Wq2,
    Wk2,
    fc1_w,
    fc1_b,
    fc2_w,
    fc2_b,
    fc3_w,
    fc3_b,
):
    global _last_results
    import ml_dtypes
    from concourse.bass_utils import run_bass_kernel_spmd

    f32 = np.float32
    bf16 = ml_dtypes.bfloat16
    fc2_w = np.asarray(fc2_w, f32)
    fc1_b = np.asarray(fc1_b, f32)
    fc2_b = np.asarray(fc2_b, f32)
    fc3_w = np.asarray(fc3_w, f32)
    fc3_b = np.asarray(fc3_b, f32)

    hhT, hnT, g1a, g2a, g1c, g2c = _host_prep(
        h_hat, pos_pickup, pos_delivery, solution, Wq1, Wk1, Wq2, Wk2,
        np.asarray(fc1_w, f32),
    )

    # block-diagonal packed MLP weights (4 independent 32-blocks)
    w2d = np.zeros((D, 128), f32)
    w3d = np.zeros((D, 4), f32)
    for r in range(4):
        w2d[32 * r : 32 * r + 32, 32 * r : 32 * r + 32] = fc2_w
        w3d[32 * r : 32 * r + 32, r : r + 1] = fc3_w.reshape(32, 1)
    b1r = np.tile(fc1_b.reshape(32, 1), (4, 1)).astype(f32)
    b2r = np.tile(fc2_b.reshape(32, 1), (4, 1)).astype(f32)

    if "nc" not in _cache:
        _cache["nc"] = _build_program()
    nc = _cache["nc"]

    in_maps = []
    for c in range(NCORES):
        bs = slice(BPC * c, BPC * (c + 1))
        in_maps.append(
            {
                "hhT": np.ascontiguousarray(hhT[bs]),
                "hnT": np.ascontiguousarray(hnT[bs]),
                "g1a": np.ascontiguousarray(g1a[bs]),
                "g2a": np.ascontiguousarray(g2a[bs]),
                "g1c": np.ascontiguousarray(g1c[bs]),
                "g2c": np.ascontiguousarray(g2c[bs]),
                "w2d": w2d.astype(bf16),
                "w3d": w3d.astype(bf16),
                "b1r": b1r,
                "b2r": b2r,
            }
        )

    res = run_bass_kernel_spmd(nc, in_maps, core_ids=list(range(NCORES)))
    _last_results = res

    out = np.concatenate(
        [res.results[c]["out"][:, :N, :] for c in range(NCORES)], axis=0
    )
    b3 = float(fc3_b.reshape(-1)[0])
    if b3 != 0.0:
        out = out + b3
    return out.astype(f32)
